# revision 11
# baseline (speedup 1.0000x reference)
"""Trainium2 Bass kernel for nn_Attention (GQA + RoPE + softmax-n + causal).

Full inputs -> shard DP2(batch) x TP4(heads) across 8 cores -> gather+sum.

Per-core device program (all matmuls fp32r, PSUM fp32):
  phase 1: Q^T/K^T/V^T = w.T @ x^T   (x^T streamed in 512-col chunks)
           RoPE on Q^T/K^T via sign-folded tables + DMA partition half-swap
           V^T transposed back to natural V via PE transpose
  phase 2: per q-chunk (512 cols), per head:
           scores^T[k,q] = K^T.T @ Q^T  (causal: N-sliced bands)
           E = exp(scores^T)  (softmax-n: no max subtraction; scores ~N(0,0.8))
           diag 128x128 blocks masked by multiplying a triangle mask
           denom[1,q] = ones.T @ E (+1 phantom logit), accumulated in PSUM
           out^T[hd,q] += V.T @ E ;  out^T *= broadcast(1/denom)
           then output projection for this q-chunk: out += oc.T @ wo_shard

Host: out[b] = sum over 4 TP shards of out_partial.
"""
import sys
import numpy as np

sys.path.insert(0, "/opt/trn_rl_repo")

import concourse.bass as bass
import concourse.bacc as bacc
import concourse.mybir as mybir
import concourse.tile as tile
from concourse import bass_utils
from concourse._compat import with_exitstack

F32 = mybir.dt.float32
F32R = mybir.dt.float32r
EXP = mybir.ActivationFunctionType.Exp

B, S, D = 2, 2048, 2048
N_HEADS, N_KV_HEADS, HD = 16, 8, 128
TP = 4                      # tensor-parallel ways (x DP2 over batch = 8 cores)
QF = 4 * HD                 # per-core q feature cols   (512)
KF = 2 * HD                 # per-core k/v feature cols (256)
NQT = S // 128              # 16 seq tiles
NQC = S // 512              # 4  q-chunks
ND = D // 128               # 16 contraction tiles
NSC = S // 512              # 4  x^T stream chunks

_CACHE = {}


def _build(bench_reps=None):
    nc = bacc.Bacc("TRN2", target_bir_lowering=False, debug=False)

    names = [("xT", [D, S], F32R), ("wq", [D, QF], F32R), ("wk", [D, KF], F32R),
             ("wv", [D, KF], F32R), ("wo", [QF, D], F32R),
             ("c2", [128, S], F32), ("g", [128, S], F32),
             ("tri", [128, 128], F32R), ("tri2", [128, 256], F32R),
             ("ones128", [128, 1], F32R),
             ("onesrow", [1, 128], F32R), ("one1", [1, 1], F32R),
             ("onerow512", [1, 512], F32R), ("idn", [128, 128], F32R)]
    kind = "Internal" if bench_reps else "ExternalInput"
    io = {n: nc.dram_tensor(n, sh, dt, kind=kind) for n, sh, dt in names}
    if bench_reps:
        io["dummy"] = nc.dram_tensor("bench_in", [128, 1], F32,
                                     kind="ExternalInput")
    io["out"] = nc.dram_tensor("out", [S, D], F32, kind="ExternalOutput")

    with tile.TileContext(nc) as tc:
        if bench_reps and bench_reps > 1:
            with tc.For_i(0, bench_reps, 1):
                _emit(tc, nc, io)
        else:
            _emit(tc, nc, io)
    nc.compile()
    return nc


@with_exitstack
def _emit(ctx, tc, nc, io):
    ts = bass.ts
    persist = ctx.enter_context(tc.tile_pool(name="persist", bufs=1))

    # ---- persistent SBUF tensors (live whole kernel) ----
    tri = persist.tile([128, 128], F32R, tag="tri")
    tri2 = persist.tile([128, 256], F32R, tag="tri2")
    ones128 = persist.tile([128, 1], F32R, tag="ones128")
    onesrow = persist.tile([1, 128], F32R, tag="onesrow")
    one1 = persist.tile([1, 1], F32R, tag="one1")
    onerow512 = persist.tile([1, 512], F32R, tag="onerow512")
    for name, t in [("tri", tri), ("tri2", tri2), ("ones128", ones128),
                    ("onesrow", onesrow), ("one1", one1),
                    ("onerow512", onerow512)]:
        nc.sync.dma_start(t[:], io[name][:])

    # rotated Q^T/K^T: 6 head tiles [128, S]; V natural: 16 tiles [128, KF]
    qkT = [persist.tile([128, S], F32R, tag=f"qkT{f}", name=f"qkT{f}")
           for f in range(6)]
    vnat = [persist.tile([128, KF], F32R, tag=f"vnat{st}", name=f"vnat{st}")
            for st in range(NQT)]
    # ================= phase 1: projections + rope + V transpose ==========
    with tc.tile_pool(name="wp", bufs=1) as wp, \
         tc.tile_pool(name="xtp", bufs=22) as xtp, \
         tc.tile_pool(name="cgp", bufs=2) as cgp, \
         tc.tile_pool(name="rope", bufs=2) as ropep, \
         tc.tile_pool(name="vsb", bufs=2) as vsbp, \
         tc.tile_pool(name="p1ps", bufs=4, space="PSUM") as p1ps, \
         tc.tile_pool(name="vtps", bufs=2, space="PSUM") as vtps:
        idn = wp.tile([128, 128], F32R, tag="idn")
        nc.sync.dma_start(idn[:], io["idn"][:])
        wq_sb = [wp.tile([128, QF], F32R, tag=f"wq{d}", name=f"wq{d}")
                 for d in range(ND)]
        wk_sb = [wp.tile([128, KF], F32R, tag=f"wk{d}", name=f"wk{d}")
                 for d in range(ND)]
        wv_sb = [wp.tile([128, KF], F32R, tag=f"wv{d}", name=f"wv{d}")
                 for d in range(ND)]
        # DMA issue order tuned to consumption order: wq f-block 0
        # interleaved with x^T chunk 0, rope tables, remaining wq blocks,
        # then wk/wv blocks. x^T chunk sc+1 is prefetched mid-chunk.
        def load_chunk(sc):
            tiles = []
            for d in range(ND):
                t = xtp.tile([128, 512], F32R, tag="xt", name=f"xt{sc}_{d}")
                nc.sync.dma_start(t[:], io["xT"][ts(d, 128), ts(sc, 512)])
                tiles.append(t)
            return tiles

        xt_next = []
        for d in range(ND):
            nc.sync.dma_start(wq_sb[d][:], io["wq"][ts(d, 128), :])
            t = xtp.tile([128, 512], F32R, tag="xt", name=f"xt0_{d}")
            nc.sync.dma_start(t[:], io["xT"][ts(d, 128), 0:512])
            xt_next.append(t)
        for d in range(ND):
            nc.sync.dma_start(wk_sb[d][:], io["wk"][ts(d, 128), :])
        for d in range(ND):
            nc.sync.dma_start(wv_sb[d][:], io["wv"][ts(d, 128), :])

        for sc in range(NSC):                    # 512-wide x^T chunks
            cs = ts(sc, 512)
            xt = xt_next
            c2c = cgp.tile([128, 512], F32, tag="c2c")
            gc = cgp.tile([128, 512], F32, tag="gc")
            nc.sync.dma_start(c2c[:], io["c2"][:, cs])
            nc.sync.dma_start(gc[:], io["g"][:, cs])
            # f: 0..3 q-heads, 4..5 k-heads, 6..7 v-heads
            for f in range(8):
                if f < 4:
                    wt, fo = wq_sb, f * 128
                elif f < 6:
                    wt, fo = wk_sb, (f - 4) * 128
                else:
                    wt, fo = wv_sb, (f - 6) * 128
                ps = p1ps.tile([128, 512], F32, tag="proj")
                for d in range(ND):
                    nc.tensor.matmul(ps[:], wt[d][:, fo:fo + 128], xt[d][:],
                                     start=(d == 0), stop=(d == ND - 1))
                if f == 0 and sc + 1 < NSC:
                    xt_next = load_chunk(sc + 1)
                if f < 6:
                    # rope: rot = ps*c2 + halfswap(ps*g)
                    a = ropep.tile([128, 512], F32, tag="ropeA")
                    b = ropep.tile([128, 512], F32, tag="ropeB")
                    bsw = ropep.tile([128, 512], F32, tag="ropeBsw")
                    nc.vector.tensor_mul(a[:], ps[:], c2c[:])
                    nc.vector.tensor_mul(b[:], ps[:], gc[:])
                    nc.sync.dma_start(bsw[0:64, :], b[64:128, :])
                    nc.sync.dma_start(bsw[64:128, :], b[0:64, :])
                    nc.vector.tensor_add(qkT[f][:, cs], a[:], bsw[:])
                else:
                    # V^T -> copy to SBUF -> PE-transpose 128x128 blocks
                    vt = vsbp.tile([128, 512], F32R, tag="vT")
                    nc.vector.tensor_copy(vt[:], ps[:])
                    for sub in range(4):
                        st = sc * 4 + sub
                        tp = vtps.tile([128, 128], F32R, tag="vtp")
                        nc.tensor.transpose(tp[:], vt[:, ts(sub, 128)], idn[:])
                        nc.vector.tensor_copy(
                            vnat[st][:, (f - 6) * 128:(f - 5) * 128], tp[:])

    # ============ phase 2: attention + fused output projection ============
    with tc.tile_pool(name="ep", bufs=4) as ep, \
         tc.tile_pool(name="ocp", bufs=2) as ocp, \
         tc.tile_pool(name="fin", bufs=2) as finp, \
         tc.tile_pool(name="osb", bufs=4) as osbp, \
         tc.tile_pool(name="scps", bufs=2, space="PSUM") as scps, \
         tc.tile_pool(name="outps", bufs=2, space="PSUM") as outps, \
         tc.tile_pool(name="denps", bufs=1, space="PSUM") as denps, \
         tc.tile_pool(name="bcps", bufs=1, space="PSUM") as bcps, \
         tc.tile_pool(name="w3ps", bufs=2, space="PSUM") as w3ps, \
         tc.tile_pool(name="wop", bufs=1) as wop:
        wo_sb = [wop.tile([128, D], F32R, tag=f"wo{hf}", name=f"wo{hf}")
                 for hf in range(4)]
        for hf in range(4):
            nc.sync.dma_start(wo_sb[hf][:], io["wo"][ts(hf, 128), :])
        for qc in range(NQC):
            qs = qc * 512
            oc = []
            for h in range(4):
                gkv = h // 2
                qT, kT = qkT[h], qkT[4 + gkv]
                out_ps = outps.tile([128, 512], F32, tag="out")
                den_ps = denps.tile([1, 512], F32, tag="den")
                # +1 phantom logit (softmax-n)
                nc.tensor.matmul(den_ps[:], one1[:], onerow512[:],
                                 start=True, stop=False)
                nkt = 4 * (qc + 1)
                for kt in range(nkt):
                    off = max(0, 128 * kt - qs)
                    diag = kt >= 4 * qc
                    moff = off
                    if off == 384:
                        off = 256        # keep N>=256 (fp32r full rate)
                    sc_ps = scps.tile([128, 512], F32, tag="sc")
                    nc.tensor.matmul(sc_ps[:, off:], kT[:, ts(kt, 128)],
                                     qT[:, qs + off:qs + 512],
                                     start=True, stop=True)
                    e = ep.tile([128, 512], F32R, tag="e")
                    nc.scalar.activation(e[:, off:], sc_ps[:, off:], EXP)
                    if diag:
                        if moff == 384:
                            nc.vector.tensor_mul(e[:, 256:512],
                                                 e[:, 256:512], tri2[:])
                        else:
                            nc.vector.tensor_mul(e[:, moff:moff + 128],
                                                 e[:, moff:moff + 128], tri[:])
                    nc.tensor.matmul(out_ps[:, off:],
                                     vnat[kt][:, gkv * 128:(gkv + 1) * 128],
                                     e[:, off:],
                                     start=(kt == 0), stop=(kt == nkt - 1))
                    nc.tensor.matmul(den_ps[:, off:], ones128[:], e[:, off:],
                                     start=False, stop=(kt == nkt - 1))
                rec = finp.tile([1, 512], F32R, tag="rec")
                with nc.allow_low_precision(reason="f32r recip feeds matmul"):
                    nc.vector.reciprocal(rec[:], den_ps[:])
                bc = bcps.tile([128, 512], F32, tag="bc")
                nc.tensor.matmul(bc[:], onesrow[:], rec[:], start=True,
                                 stop=True)
                bcs = finp.tile([128, 512], F32, tag="bcs")
                nc.vector.tensor_copy(bcs[:], bc[:])
                o = ocp.tile([128, 512], F32R, tag=f"oc{h}", name=f"oc{h}_{qc}")
                nc.vector.tensor_mul(o[:], out_ps[:], bcs[:])
                oc.append(o)
            # fused output projection for this q-chunk's 4 seq tiles
            for sub in range(4):
                st = qc * 4 + sub
                for dc in range(4):
                    ps3 = w3ps.tile([128, 512], F32, tag="wo3")
                    for hf in range(4):
                        nc.tensor.matmul(ps3[:], oc[hf][:, ts(sub, 128)],
                                         wo_sb[hf][:, ts(dc, 512)],
                                         start=(hf == 0), stop=(hf == 3))
                    o3 = osbp.tile([128, 512], F32, tag="o3")
                    nc.vector.tensor_copy(o3[:], ps3[:])
                    nc.sync.dma_start(io["out"][ts(st, 128), ts(dc, 512)], o3[:])


def _host_prep(x, freqs_cos, freqs_sin, wq, wk, wv, wo):
    """Build the 8 per-core input maps."""
    # de-interleave perm within every 128-col head block: [0,2,..,126,1,3,..,127]
    p128 = np.concatenate([np.arange(0, 128, 2), np.arange(1, 128, 2)])
    permq = np.concatenate([hb * 128 + p128 for hb in range(N_HEADS)])
    permk = np.concatenate([hb * 128 + p128 for hb in range(N_KV_HEADS)])
    wq_p = (wq / np.sqrt(np.float32(HD)))[:, permq]
    wk_p = wk[:, permk]

    cosT = np.ascontiguousarray(freqs_cos.T)            # [64, S]
    sinT = np.ascontiguousarray(freqs_sin.T)
    c2 = np.concatenate([cosT, cosT], 0).astype(np.float32)   # [128, S]
    gtab = np.concatenate([sinT, -sinT], 0).astype(np.float32)

    ii, jj = np.meshgrid(np.arange(128), np.arange(128), indexing="ij")
    tri = (ii <= jj).astype(np.float32)                 # [k, q] allow k<=q

    tri2 = np.concatenate([np.zeros((128, 128), np.float32), tri], 1)
    common = {
        "c2": c2, "g": gtab, "tri": tri, "tri2": tri2,
        "ones128": np.ones((128, 1), np.float32),
        "onesrow": np.ones((1, 128), np.float32),
        "one1": np.ones((1, 1), np.float32),
        "onerow512": np.ones((1, 512), np.float32),
        "idn": np.eye(128, dtype=np.float32),
    }
    in_maps = []
    for core in range(8):
        b, t = divmod(core, TP)
        in_maps.append({
            "xT": np.ascontiguousarray(x[b].T).astype(np.float32),
            "wq": np.ascontiguousarray(wq_p[:, t * QF:(t + 1) * QF]),
            "wk": np.ascontiguousarray(wk_p[:, t * KF:(t + 1) * KF]),
            "wv": np.ascontiguousarray(wv[:, t * KF:(t + 1) * KF]),
            "wo": np.ascontiguousarray(wo[t * QF:(t + 1) * QF, :]),
            **common,
        })
    return in_maps


def kernel(x, freqs_cos, freqs_sin, wq, wk, wv, wo, _trace=False):
    in_maps = _host_prep(np.asarray(x, np.float32),
                         np.asarray(freqs_cos, np.float32),
                         np.asarray(freqs_sin, np.float32),
                         np.asarray(wq, np.float32), np.asarray(wk, np.float32),
                         np.asarray(wv, np.float32), np.asarray(wo, np.float32))
    if "nc" not in _CACHE:
        _CACHE["nc"] = _build()
    res = bass_utils.run_bass_kernel_spmd(_CACHE["nc"], in_maps, list(range(8)),
                                          trace=_trace)
    _CACHE["last_result"] = res
    out = np.zeros((B, S, D), np.float32)
    for core in range(8):
        b = core // TP
        out[b] += res.results[core]["out"]
    return out


# revision 12
# speedup vs baseline: 1.1940x; 1.1940x over previous
"""Trainium2 Bass kernel for nn_Attention (GQA + RoPE + softmax-n + causal).

Full inputs -> shard DP2(batch) x TP4(heads) across 8 cores -> gather+sum.

Per-core device program (all matmuls fp32r, PSUM fp32):
  phase 1: Q^T/K^T/V^T = w.T @ x^T   (x^T streamed in 512-col chunks)
           RoPE on Q^T/K^T via sign-folded tables + DMA partition half-swap
           V^T transposed back to natural V via PE transpose
  phase 2: per q-chunk (512 cols), per head:
           scores^T[k,q] = K^T.T @ Q^T  (causal: N-sliced bands)
           E = exp(scores^T)  (softmax-n: no max subtraction; scores ~N(0,0.8))
           diag 128x128 blocks masked by multiplying a triangle mask
           denom[1,q] = ones.T @ E (+1 phantom logit), accumulated in PSUM
           out^T[hd,q] += V.T @ E ;  out^T *= broadcast(1/denom)
           then output projection for this q-chunk: out += oc.T @ wo_shard

Host: out[b] = sum over 4 TP shards of out_partial.
"""
import sys
import numpy as np

sys.path.insert(0, "/opt/trn_rl_repo")

import concourse.bass as bass
import concourse.bacc as bacc
import concourse.mybir as mybir
import concourse.tile as tile
from concourse import bass_utils
from concourse._compat import with_exitstack

F32 = mybir.dt.float32
F32R = mybir.dt.float32r
EXP = mybir.ActivationFunctionType.Exp

B, S, D = 2, 2048, 2048
N_HEADS, N_KV_HEADS, HD = 16, 8, 128
TP = 4                      # tensor-parallel ways (x DP2 over batch = 8 cores)
QF = 4 * HD                 # per-core q feature cols   (512)
KF = 2 * HD                 # per-core k/v feature cols (256)
NQT = S // 128              # 16 seq tiles
NQC = S // 512              # 4  q-chunks
ND = D // 128               # 16 contraction tiles
NSC = S // 512              # 4  x^T stream chunks

_CACHE = {}


def _build(bench_reps=None):
    nc = bacc.Bacc("TRN2", target_bir_lowering=False, debug=False)

    names = [("xT", [D, S], F32R), ("wq", [D, QF], F32R), ("wk", [D, KF], F32R),
             ("wv", [D, KF], F32R), ("wo", [QF, D], F32R),
             ("c2", [128, S], F32), ("g", [128, S], F32),
             ("tri", [128, 128], F32R), ("tri2", [128, 256], F32R),
             ("ones128", [128, 1], F32R),
             ("onesrow", [1, 128], F32R), ("one1", [1, 1], F32R),
             ("onerow512", [1, 512], F32R), ("idn", [128, 128], F32R)]
    kind = "Internal" if bench_reps else "ExternalInput"
    io = {n: nc.dram_tensor(n, sh, dt, kind=kind) for n, sh, dt in names}
    if bench_reps:
        io["dummy"] = nc.dram_tensor("bench_in", [128, 1], F32,
                                     kind="ExternalInput")
    io["out"] = nc.dram_tensor("out", [S, D], F32, kind="ExternalOutput")

    with tile.TileContext(nc) as tc:
        if bench_reps and bench_reps > 1:
            with tc.For_i(0, bench_reps, 1):
                _emit(tc, nc, io)
        else:
            _emit(tc, nc, io)
    nc.compile()
    return nc


@with_exitstack
def _emit(ctx, tc, nc, io):
    ts = bass.ts
    persist = ctx.enter_context(tc.tile_pool(name="persist", bufs=1))

    # ---- persistent SBUF tensors (live whole kernel) ----
    tri = persist.tile([128, 128], F32R, tag="tri")
    tri2 = persist.tile([128, 256], F32R, tag="tri2")
    ones128 = persist.tile([128, 1], F32R, tag="ones128")
    onesrow = persist.tile([1, 128], F32R, tag="onesrow")
    one1 = persist.tile([1, 1], F32R, tag="one1")
    onerow512 = persist.tile([1, 512], F32R, tag="onerow512")
    for name, t in [("tri", tri), ("tri2", tri2), ("ones128", ones128),
                    ("onesrow", onesrow), ("one1", one1),
                    ("onerow512", onerow512)]:
        nc.sync.dma_start(t[:], io[name][:])

    # rotated Q^T/K^T: 6 head tiles [128, S]; V natural: 16 tiles [128, KF]
    qkT = [persist.tile([128, S], F32R, tag=f"qkT{f}", name=f"qkT{f}")
           for f in range(6)]
    vnat = [persist.tile([128, KF], F32R, tag=f"vnat{st}", name=f"vnat{st}")
            for st in range(NQT)]
    # ================= phase 1: projections + rope + V transpose ==========
    with tc.tile_pool(name="wp", bufs=1) as wp, \
         tc.tile_pool(name="xtp", bufs=22) as xtp, \
         tc.tile_pool(name="cgp", bufs=2) as cgp, \
         tc.tile_pool(name="rope", bufs=2) as ropep, \
         tc.tile_pool(name="vsb", bufs=2) as vsbp, \
         tc.tile_pool(name="p1ps", bufs=4, space="PSUM") as p1ps, \
         tc.tile_pool(name="vtps", bufs=2, space="PSUM") as vtps:
        idn = wp.tile([128, 128], F32R, tag="idn")
        nc.sync.dma_start(idn[:], io["idn"][:])
        wq_sb = [wp.tile([128, QF], F32R, tag=f"wq{d}", name=f"wq{d}")
                 for d in range(ND)]
        wk_sb = [wp.tile([128, KF], F32R, tag=f"wk{d}", name=f"wk{d}")
                 for d in range(ND)]
        wv_sb = [wp.tile([128, KF], F32R, tag=f"wv{d}", name=f"wv{d}")
                 for d in range(ND)]
        # DMA issue order tuned to consumption order: wq f-block 0
        # interleaved with x^T chunk 0, rope tables, remaining wq blocks,
        # then wk/wv blocks. x^T chunk sc+1 is prefetched mid-chunk.
        def load_chunk(sc):
            tiles = []
            for d in range(ND):
                t = xtp.tile([128, 512], F32R, tag="xt", name=f"xt{sc}_{d}")
                nc.scalar.dma_start(t[:], io["xT"][ts(d, 128), ts(sc, 512)])
                tiles.append(t)
            return tiles

        xt_next = []
        for d in range(ND):
            nc.sync.dma_start(wq_sb[d][:], io["wq"][ts(d, 128), :])
            t = xtp.tile([128, 512], F32R, tag="xt", name=f"xt0_{d}")
            nc.scalar.dma_start(t[:], io["xT"][ts(d, 128), 0:512])
            xt_next.append(t)
        for d in range(ND):
            nc.sync.dma_start(wk_sb[d][:], io["wk"][ts(d, 128), :])
        for d in range(ND):
            nc.sync.dma_start(wv_sb[d][:], io["wv"][ts(d, 128), :])

        for sc in range(NSC):                    # 512-wide x^T chunks
            cs = ts(sc, 512)
            xt = xt_next
            c2c = cgp.tile([128, 512], F32, tag="c2c")
            gc = cgp.tile([128, 512], F32, tag="gc")
            nc.sync.dma_start(c2c[:], io["c2"][:, cs])
            nc.sync.dma_start(gc[:], io["g"][:, cs])
            # f: 0..3 q-heads, 4..5 k-heads, 6..7 v-heads
            for f in range(8):
                if f < 4:
                    wt, fo = wq_sb, f * 128
                elif f < 6:
                    wt, fo = wk_sb, (f - 4) * 128
                else:
                    wt, fo = wv_sb, (f - 6) * 128
                ps = p1ps.tile([128, 512], F32, tag="proj")
                for d in range(ND):
                    nc.tensor.matmul(ps[:], wt[d][:, fo:fo + 128], xt[d][:],
                                     start=(d == 0), stop=(d == ND - 1))
                if f == 0 and sc + 1 < NSC:
                    xt_next = load_chunk(sc + 1)
                if f < 6:
                    # rope: rot = ps*c2 + halfswap(ps*g)
                    a = ropep.tile([128, 512], F32, tag="ropeA")
                    b = ropep.tile([128, 512], F32, tag="ropeB")
                    bsw = ropep.tile([128, 512], F32, tag="ropeBsw")
                    nc.vector.tensor_mul(a[:], ps[:], c2c[:])
                    nc.vector.tensor_mul(b[:], ps[:], gc[:])
                    nc.sync.dma_start(bsw[0:64, :], b[64:128, :])
                    nc.sync.dma_start(bsw[64:128, :], b[0:64, :])
                    nc.vector.tensor_add(qkT[f][:, cs], a[:], bsw[:])
                else:
                    # V^T -> copy to SBUF -> PE-transpose 128x128 blocks
                    vt = vsbp.tile([128, 512], F32R, tag="vT")
                    nc.vector.tensor_copy(vt[:], ps[:])
                    for sub in range(4):
                        st = sc * 4 + sub
                        tp = vtps.tile([128, 128], F32R, tag="vtp")
                        nc.tensor.transpose(tp[:], vt[:, ts(sub, 128)], idn[:])
                        nc.vector.tensor_copy(
                            vnat[st][:, (f - 6) * 128:(f - 5) * 128], tp[:])

    # ============ phase 2: attention + fused output projection ============
    with tc.tile_pool(name="ep", bufs=4) as ep, \
         tc.tile_pool(name="ocp", bufs=2) as ocp, \
         tc.tile_pool(name="fin", bufs=2) as finp, \
         tc.tile_pool(name="osb", bufs=4) as osbp, \
         tc.tile_pool(name="scps", bufs=2, space="PSUM") as scps, \
         tc.tile_pool(name="outps", bufs=2, space="PSUM") as outps, \
         tc.tile_pool(name="denps", bufs=1, space="PSUM") as denps, \
         tc.tile_pool(name="bcps", bufs=1, space="PSUM") as bcps, \
         tc.tile_pool(name="w3ps", bufs=2, space="PSUM") as w3ps, \
         tc.tile_pool(name="wop", bufs=1) as wop:
        wo_sb = [wop.tile([128, D], F32R, tag=f"wo{hf}", name=f"wo{hf}")
                 for hf in range(4)]
        for hf in range(4):
            nc.sync.dma_start(wo_sb[hf][:], io["wo"][ts(hf, 128), :])
        for qc in range(NQC):
            qs = qc * 512
            oc = []
            for h in range(4):
                gkv = h // 2
                qT, kT = qkT[h], qkT[4 + gkv]
                out_ps = outps.tile([128, 512], F32, tag="out")
                den_ps = denps.tile([1, 512], F32, tag="den")
                # +1 phantom logit (softmax-n)
                nc.tensor.matmul(den_ps[:], one1[:], onerow512[:],
                                 start=True, stop=False)
                nkt = 4 * (qc + 1)
                for kt in range(nkt):
                    off = max(0, 128 * kt - qs)
                    diag = kt >= 4 * qc
                    moff = off
                    if off == 384:
                        off = 256        # keep N>=256 (fp32r full rate)
                    sc_ps = scps.tile([128, 512], F32, tag="sc")
                    nc.tensor.matmul(sc_ps[:, off:], kT[:, ts(kt, 128)],
                                     qT[:, qs + off:qs + 512],
                                     start=True, stop=True)
                    e = ep.tile([128, 512], F32R, tag="e")
                    nc.scalar.activation(e[:, off:], sc_ps[:, off:], EXP)
                    if diag:
                        if moff == 384:
                            nc.vector.tensor_mul(e[:, 256:512],
                                                 e[:, 256:512], tri2[:])
                        else:
                            nc.vector.tensor_mul(e[:, moff:moff + 128],
                                                 e[:, moff:moff + 128], tri[:])
                    nc.tensor.matmul(out_ps[:, off:],
                                     vnat[kt][:, gkv * 128:(gkv + 1) * 128],
                                     e[:, off:],
                                     start=(kt == 0), stop=(kt == nkt - 1))
                    nc.tensor.matmul(den_ps[:, off:], ones128[:], e[:, off:],
                                     start=False, stop=(kt == nkt - 1))
                rec = finp.tile([1, 512], F32R, tag="rec")
                with nc.allow_low_precision(reason="f32r recip feeds matmul"):
                    nc.vector.reciprocal(rec[:], den_ps[:])
                bc = bcps.tile([128, 512], F32, tag="bc")
                nc.tensor.matmul(bc[:], onesrow[:], rec[:], start=True,
                                 stop=True)
                bcs = finp.tile([128, 512], F32, tag="bcs")
                nc.vector.tensor_copy(bcs[:], bc[:])
                o = ocp.tile([128, 512], F32R, tag=f"oc{h}", name=f"oc{h}_{qc}")
                nc.vector.tensor_mul(o[:], out_ps[:], bcs[:])
                oc.append(o)
            # fused output projection for this q-chunk's 4 seq tiles
            for sub in range(4):
                st = qc * 4 + sub
                for dc in range(4):
                    ps3 = w3ps.tile([128, 512], F32, tag="wo3")
                    for hf in range(4):
                        nc.tensor.matmul(ps3[:], oc[hf][:, ts(sub, 128)],
                                         wo_sb[hf][:, ts(dc, 512)],
                                         start=(hf == 0), stop=(hf == 3))
                    o3 = osbp.tile([128, 512], F32, tag="o3")
                    nc.vector.tensor_copy(o3[:], ps3[:])
                    nc.sync.dma_start(io["out"][ts(st, 128), ts(dc, 512)], o3[:])


def _host_prep(x, freqs_cos, freqs_sin, wq, wk, wv, wo):
    """Build the 8 per-core input maps."""
    # de-interleave perm within every 128-col head block: [0,2,..,126,1,3,..,127]
    p128 = np.concatenate([np.arange(0, 128, 2), np.arange(1, 128, 2)])
    permq = np.concatenate([hb * 128 + p128 for hb in range(N_HEADS)])
    permk = np.concatenate([hb * 128 + p128 for hb in range(N_KV_HEADS)])
    wq_p = (wq / np.sqrt(np.float32(HD)))[:, permq]
    wk_p = wk[:, permk]

    cosT = np.ascontiguousarray(freqs_cos.T)            # [64, S]
    sinT = np.ascontiguousarray(freqs_sin.T)
    c2 = np.concatenate([cosT, cosT], 0).astype(np.float32)   # [128, S]
    gtab = np.concatenate([sinT, -sinT], 0).astype(np.float32)

    ii, jj = np.meshgrid(np.arange(128), np.arange(128), indexing="ij")
    tri = (ii <= jj).astype(np.float32)                 # [k, q] allow k<=q

    tri2 = np.concatenate([np.zeros((128, 128), np.float32), tri], 1)
    common = {
        "c2": c2, "g": gtab, "tri": tri, "tri2": tri2,
        "ones128": np.ones((128, 1), np.float32),
        "onesrow": np.ones((1, 128), np.float32),
        "one1": np.ones((1, 1), np.float32),
        "onerow512": np.ones((1, 512), np.float32),
        "idn": np.eye(128, dtype=np.float32),
    }
    in_maps = []
    for core in range(8):
        b, t = divmod(core, TP)
        in_maps.append({
            "xT": np.ascontiguousarray(x[b].T).astype(np.float32),
            "wq": np.ascontiguousarray(wq_p[:, t * QF:(t + 1) * QF]),
            "wk": np.ascontiguousarray(wk_p[:, t * KF:(t + 1) * KF]),
            "wv": np.ascontiguousarray(wv[:, t * KF:(t + 1) * KF]),
            "wo": np.ascontiguousarray(wo[t * QF:(t + 1) * QF, :]),
            **common,
        })
    return in_maps


def kernel(x, freqs_cos, freqs_sin, wq, wk, wv, wo, _trace=False):
    in_maps = _host_prep(np.asarray(x, np.float32),
                         np.asarray(freqs_cos, np.float32),
                         np.asarray(freqs_sin, np.float32),
                         np.asarray(wq, np.float32), np.asarray(wk, np.float32),
                         np.asarray(wv, np.float32), np.asarray(wo, np.float32))
    if "nc" not in _CACHE:
        _CACHE["nc"] = _build()
    res = bass_utils.run_bass_kernel_spmd(_CACHE["nc"], in_maps, list(range(8)),
                                          trace=_trace)
    _CACHE["last_result"] = res
    out = np.zeros((B, S, D), np.float32)
    for core in range(8):
        b = core // TP
        out[b] += res.results[core]["out"]
    return out


# revision 13
# speedup vs baseline: 1.2754x; 1.0681x over previous
"""Trainium2 Bass kernel for nn_Attention (GQA + RoPE + softmax-n + causal).

Full inputs -> shard DP2(batch) x TP4(heads) across 8 cores -> gather+sum.

Per-core device program (all matmuls fp32r, PSUM fp32):
  phase 1: Q^T/K^T/V^T = w.T @ x^T   (x^T streamed in 512-col chunks)
           RoPE on Q^T/K^T via sign-folded tables + DMA partition half-swap
           V^T transposed back to natural V via PE transpose
  phase 2: per q-chunk (512 cols), per head:
           scores^T[k,q] = K^T.T @ Q^T  (causal: N-sliced bands)
           E = exp(scores^T)  (softmax-n: no max subtraction; scores ~N(0,0.8))
           diag 128x128 blocks masked by multiplying a triangle mask
           denom[1,q] = ones.T @ E (+1 phantom logit), accumulated in PSUM
           out^T[hd,q] += V.T @ E ;  out^T *= broadcast(1/denom)
           then output projection for this q-chunk: out += oc.T @ wo_shard

Host: out[b] = sum over 4 TP shards of out_partial.
"""
import sys
import numpy as np

sys.path.insert(0, "/opt/trn_rl_repo")

import concourse.bass as bass
import concourse.bacc as bacc
import concourse.mybir as mybir
import concourse.tile as tile
from concourse import bass_utils
from concourse._compat import with_exitstack

F32 = mybir.dt.float32
F32R = mybir.dt.float32r
EXP = mybir.ActivationFunctionType.Exp

B, S, D = 2, 2048, 2048
N_HEADS, N_KV_HEADS, HD = 16, 8, 128
TP = 4                      # tensor-parallel ways (x DP2 over batch = 8 cores)
QF = 4 * HD                 # per-core q feature cols   (512)
KF = 2 * HD                 # per-core k/v feature cols (256)
NQT = S // 128              # 16 seq tiles
NQC = S // 512              # 4  q-chunks
ND = D // 128               # 16 contraction tiles
NSC = S // 512              # 4  x^T stream chunks

_CACHE = {}


def _build(bench_reps=None):
    nc = bacc.Bacc("TRN2", target_bir_lowering=False, debug=False)

    names = [("xT", [D, S], F32R), ("wq", [D, QF], F32R), ("wk", [D, KF], F32R),
             ("wv", [D, KF], F32R), ("wo", [QF, D], F32R),
             ("c2", [128, S], F32), ("g", [128, S], F32),
             ("tri", [128, 128], F32R), ("tri2", [128, 256], F32R),
             ("ones128", [128, 1], F32R),
             ("onesrow", [1, 128], F32R), ("one1", [1, 1], F32R),
             ("onerow512", [1, 512], F32R), ("idn", [128, 128], F32R)]
    kind = "Internal" if bench_reps else "ExternalInput"
    io = {n: nc.dram_tensor(n, sh, dt, kind=kind) for n, sh, dt in names}
    if bench_reps:
        io["dummy"] = nc.dram_tensor("bench_in", [128, 1], F32,
                                     kind="ExternalInput")
    io["out"] = nc.dram_tensor("out", [S, D], F32, kind="ExternalOutput")

    with tile.TileContext(nc) as tc:
        if bench_reps and bench_reps > 1:
            with tc.For_i(0, bench_reps, 1):
                _emit(tc, nc, io)
        else:
            _emit(tc, nc, io)
    nc.compile()
    return nc


@with_exitstack
def _emit(ctx, tc, nc, io):
    ts = bass.ts
    persist = ctx.enter_context(tc.tile_pool(name="persist", bufs=1))

    # ---- persistent SBUF tensors (live whole kernel) ----
    tri = persist.tile([128, 128], F32R, tag="tri")
    tri2 = persist.tile([128, 256], F32R, tag="tri2")
    ones128 = persist.tile([128, 1], F32R, tag="ones128")
    onesrow = persist.tile([1, 128], F32R, tag="onesrow")
    one1 = persist.tile([1, 1], F32R, tag="one1")
    onerow512 = persist.tile([1, 512], F32R, tag="onerow512")
    for name, t in [("tri", tri), ("tri2", tri2), ("ones128", ones128),
                    ("onesrow", onesrow), ("one1", one1),
                    ("onerow512", onerow512)]:
        nc.sync.dma_start(t[:], io[name][:])

    # rotated Q^T/K^T: 6 head tiles [128, S]; V natural: 16 tiles [128, KF]
    qkT = [persist.tile([128, S], F32R, tag=f"qkT{f}", name=f"qkT{f}")
           for f in range(6)]
    vnat = [persist.tile([128, KF], F32R, tag=f"vnat{st}", name=f"vnat{st}")
            for st in range(NQT)]
    # ================= phase 1: projections + rope + V transpose ==========
    with tc.tile_pool(name="wp", bufs=1) as wp, \
         tc.tile_pool(name="xtp", bufs=22) as xtp, \
         tc.tile_pool(name="cgp", bufs=2) as cgp, \
         tc.tile_pool(name="rope", bufs=2) as ropep, \
         tc.tile_pool(name="vsb", bufs=2) as vsbp, \
         tc.tile_pool(name="p1ps", bufs=4, space="PSUM") as p1ps, \
         tc.tile_pool(name="vtps", bufs=2, space="PSUM") as vtps:
        idn = wp.tile([128, 128], F32R, tag="idn")
        nc.sync.dma_start(idn[:], io["idn"][:])
        wq_sb = [wp.tile([128, QF], F32R, tag=f"wq{d}", name=f"wq{d}")
                 for d in range(ND)]
        wk_sb = [wp.tile([128, KF], F32R, tag=f"wk{d}", name=f"wk{d}")
                 for d in range(ND)]
        wv_sb = [wp.tile([128, KF], F32R, tag=f"wv{d}", name=f"wv{d}")
                 for d in range(ND)]
        # interleave wq with x^T chunk 0 so the first matmul chain starts
        # as early as possible; wk/wv (needed later) load after.
        xt0 = []
        for d in range(ND):
            nc.sync.dma_start(wq_sb[d][:], io["wq"][ts(d, 128), :])
            t = xtp.tile([128, 512], F32R, tag="xt", name=f"xt0_{d}")
            nc.sync.dma_start(t[:], io["xT"][ts(d, 128), 0:512])
            xt0.append(t)
        for d in range(ND):
            nc.sync.dma_start(wk_sb[d][:], io["wk"][ts(d, 128), :])
            nc.sync.dma_start(wv_sb[d][:], io["wv"][ts(d, 128), :])

        for sc in range(NSC):                    # 512-wide x^T chunks
            cs = ts(sc, 512)
            if sc == 0:
                xt = xt0
            else:
                xt = []
                for d in range(ND):
                    t = xtp.tile([128, 512], F32R, tag="xt", name=f"xt{sc}_{d}")
                    nc.sync.dma_start(t[:], io["xT"][ts(d, 128), cs])
                    xt.append(t)
            c2c = cgp.tile([128, 512], F32, tag="c2c")
            gc = cgp.tile([128, 512], F32, tag="gc")
            nc.sync.dma_start(c2c[:], io["c2"][:, cs])
            nc.sync.dma_start(gc[:], io["g"][:, cs])
            # f: 0..3 q-heads, 4..5 k-heads, 6..7 v-heads
            for f in range(8):
                if f < 4:
                    wt, fo = wq_sb, f * 128
                elif f < 6:
                    wt, fo = wk_sb, (f - 4) * 128
                else:
                    wt, fo = wv_sb, (f - 6) * 128
                ps = p1ps.tile([128, 512], F32, tag="proj")
                for d in range(ND):
                    nc.tensor.matmul(ps[:], wt[d][:, fo:fo + 128], xt[d][:],
                                     start=(d == 0), stop=(d == ND - 1))
                if f < 6:
                    # rope: rot = ps*c2 + halfswap(ps*g)
                    a = ropep.tile([128, 512], F32, tag="ropeA")
                    b = ropep.tile([128, 512], F32, tag="ropeB")
                    bsw = ropep.tile([128, 512], F32, tag="ropeBsw")
                    nc.vector.tensor_mul(a[:], ps[:], c2c[:])
                    nc.vector.tensor_mul(b[:], ps[:], gc[:])
                    nc.sync.dma_start(bsw[0:64, :], b[64:128, :])
                    nc.sync.dma_start(bsw[64:128, :], b[0:64, :])
                    nc.vector.tensor_add(qkT[f][:, cs], a[:], bsw[:])
                else:
                    # V^T -> copy to SBUF -> PE-transpose 128x128 blocks
                    vt = vsbp.tile([128, 512], F32R, tag="vT")
                    nc.vector.tensor_copy(vt[:], ps[:])
                    for sub in range(4):
                        st = sc * 4 + sub
                        tp = vtps.tile([128, 128], F32R, tag="vtp")
                        nc.tensor.transpose(tp[:], vt[:, ts(sub, 128)], idn[:])
                        nc.vector.tensor_copy(
                            vnat[st][:, (f - 6) * 128:(f - 5) * 128], tp[:])

    # ============ phase 2: attention + fused output projection ============
    with tc.tile_pool(name="ep", bufs=4) as ep, \
         tc.tile_pool(name="ocp", bufs=2) as ocp, \
         tc.tile_pool(name="fin", bufs=2) as finp, \
         tc.tile_pool(name="osb", bufs=4) as osbp, \
         tc.tile_pool(name="scps", bufs=2, space="PSUM") as scps, \
         tc.tile_pool(name="outps", bufs=2, space="PSUM") as outps, \
         tc.tile_pool(name="denps", bufs=1, space="PSUM") as denps, \
         tc.tile_pool(name="bcps", bufs=1, space="PSUM") as bcps, \
         tc.tile_pool(name="w3ps", bufs=2, space="PSUM") as w3ps, \
         tc.tile_pool(name="wop", bufs=1) as wop:
        wo_sb = [wop.tile([128, D], F32R, tag=f"wo{hf}", name=f"wo{hf}")
                 for hf in range(4)]
        for hf in range(4):
            nc.sync.dma_start(wo_sb[hf][:], io["wo"][ts(hf, 128), :])
        for qc in range(NQC):
            qs = qc * 512
            oc = []
            for h in range(4):
                gkv = h // 2
                qT, kT = qkT[h], qkT[4 + gkv]
                out_ps = outps.tile([128, 512], F32, tag="out")
                den_ps = denps.tile([1, 512], F32, tag="den")
                # +1 phantom logit (softmax-n)
                nc.tensor.matmul(den_ps[:], one1[:], onerow512[:],
                                 start=True, stop=False)
                nkt = 4 * (qc + 1)
                for kt in range(nkt):
                    off = max(0, 128 * kt - qs)
                    diag = kt >= 4 * qc
                    moff = off
                    if off == 384:
                        off = 256        # keep N>=256 (fp32r full rate)
                    sc_ps = scps.tile([128, 512], F32, tag="sc")
                    nc.tensor.matmul(sc_ps[:, off:], kT[:, ts(kt, 128)],
                                     qT[:, qs + off:qs + 512],
                                     start=True, stop=True)
                    e = ep.tile([128, 512], F32R, tag="e")
                    nc.scalar.activation(e[:, off:], sc_ps[:, off:], EXP)
                    if diag:
                        if moff == 384:
                            nc.vector.tensor_mul(e[:, 256:512],
                                                 e[:, 256:512], tri2[:])
                        else:
                            nc.vector.tensor_mul(e[:, moff:moff + 128],
                                                 e[:, moff:moff + 128], tri[:])
                    nc.tensor.matmul(out_ps[:, off:],
                                     vnat[kt][:, gkv * 128:(gkv + 1) * 128],
                                     e[:, off:],
                                     start=(kt == 0), stop=(kt == nkt - 1))
                    nc.tensor.matmul(den_ps[:, off:], ones128[:], e[:, off:],
                                     start=False, stop=(kt == nkt - 1))
                rec = finp.tile([1, 512], F32R, tag="rec")
                with nc.allow_low_precision(reason="f32r recip feeds matmul"):
                    nc.vector.reciprocal(rec[:], den_ps[:])
                bc = bcps.tile([128, 512], F32, tag="bc")
                nc.tensor.matmul(bc[:], onesrow[:], rec[:], start=True,
                                 stop=True)
                bcs = finp.tile([128, 512], F32, tag="bcs")
                nc.vector.tensor_copy(bcs[:], bc[:])
                o = ocp.tile([128, 512], F32R, tag=f"oc{h}", name=f"oc{h}_{qc}")
                nc.vector.tensor_mul(o[:], out_ps[:], bcs[:])
                oc.append(o)
            # fused output projection for this q-chunk's 4 seq tiles
            for sub in range(4):
                st = qc * 4 + sub
                for dc in range(4):
                    ps3 = w3ps.tile([128, 512], F32, tag="wo3")
                    for hf in range(4):
                        nc.tensor.matmul(ps3[:], oc[hf][:, ts(sub, 128)],
                                         wo_sb[hf][:, ts(dc, 512)],
                                         start=(hf == 0), stop=(hf == 3))
                    o3 = osbp.tile([128, 512], F32, tag="o3")
                    nc.vector.tensor_copy(o3[:], ps3[:])
                    nc.sync.dma_start(io["out"][ts(st, 128), ts(dc, 512)], o3[:])


def _host_prep(x, freqs_cos, freqs_sin, wq, wk, wv, wo):
    """Build the 8 per-core input maps."""
    # de-interleave perm within every 128-col head block: [0,2,..,126,1,3,..,127]
    p128 = np.concatenate([np.arange(0, 128, 2), np.arange(1, 128, 2)])
    permq = np.concatenate([hb * 128 + p128 for hb in range(N_HEADS)])
    permk = np.concatenate([hb * 128 + p128 for hb in range(N_KV_HEADS)])
    wq_p = (wq / np.sqrt(np.float32(HD)))[:, permq]
    wk_p = wk[:, permk]

    cosT = np.ascontiguousarray(freqs_cos.T)            # [64, S]
    sinT = np.ascontiguousarray(freqs_sin.T)
    c2 = np.concatenate([cosT, cosT], 0).astype(np.float32)   # [128, S]
    gtab = np.concatenate([sinT, -sinT], 0).astype(np.float32)

    ii, jj = np.meshgrid(np.arange(128), np.arange(128), indexing="ij")
    tri = (ii <= jj).astype(np.float32)                 # [k, q] allow k<=q

    tri2 = np.concatenate([np.zeros((128, 128), np.float32), tri], 1)
    common = {
        "c2": c2, "g": gtab, "tri": tri, "tri2": tri2,
        "ones128": np.ones((128, 1), np.float32),
        "onesrow": np.ones((1, 128), np.float32),
        "one1": np.ones((1, 1), np.float32),
        "onerow512": np.ones((1, 512), np.float32),
        "idn": np.eye(128, dtype=np.float32),
    }
    in_maps = []
    for core in range(8):
        b, t = divmod(core, TP)
        in_maps.append({
            "xT": np.ascontiguousarray(x[b].T).astype(np.float32),
            "wq": np.ascontiguousarray(wq_p[:, t * QF:(t + 1) * QF]),
            "wk": np.ascontiguousarray(wk_p[:, t * KF:(t + 1) * KF]),
            "wv": np.ascontiguousarray(wv[:, t * KF:(t + 1) * KF]),
            "wo": np.ascontiguousarray(wo[t * QF:(t + 1) * QF, :]),
            **common,
        })
    return in_maps


def kernel(x, freqs_cos, freqs_sin, wq, wk, wv, wo, _trace=False):
    in_maps = _host_prep(np.asarray(x, np.float32),
                         np.asarray(freqs_cos, np.float32),
                         np.asarray(freqs_sin, np.float32),
                         np.asarray(wq, np.float32), np.asarray(wk, np.float32),
                         np.asarray(wv, np.float32), np.asarray(wo, np.float32))
    if "nc" not in _CACHE:
        _CACHE["nc"] = _build()
    res = bass_utils.run_bass_kernel_spmd(_CACHE["nc"], in_maps, list(range(8)),
                                          trace=_trace)
    _CACHE["last_result"] = res
    out = np.zeros((B, S, D), np.float32)
    for core in range(8):
        b = core // TP
        out[b] += res.results[core]["out"]
    return out
